# revision 13
# baseline (speedup 1.0000x reference)
"""Trainium2 Bass kernel for nn_CrossBaby_1 (B=32, S=128, V=8192, E=256).

Strategy (8 NeuronCores, single NEFF, collectives, 2-deep software pipeline):
  - Step 1 (x @ w_emb.T, 17 GFLOP): data-parallel over batch. Each core
    computes hT for its 4 batches from a host-pretransposed, centered
    (x-0.5) shard in fp8-e3m4. w_emb is also fp8-e3m4, host-scaled by 64
    (undone by the activation's scale). x + w_emb stream in lockstep on
    the sync HWDGE ring; wr/w2/wo stream right behind them, all during
    phase 1's compute. Dependent DMAs (gather bounce, readbacks, output)
    live on the gpsimd (SWDGE) queue so the input ring never stalls.
  - ONE merged AllGather of (hT bf16 + row-sums s) across the 8 cores.
  - Steps 3-5 (w_red / w_red2): tensor-parallel over the e/j feature dim.
  - y2 partial reduction: AllGather + local DVE tree-sum.
  - Step 6 (w_out): tensor-parallel over vocab; host concatenates.
  - Bias adds for steps 4/6 are folded into the PSUM accumulation as
    rank-1 matmuls (ones ⊗ bias_row).
  - The rep loop is software-pipelined two deep: body(i) emits
    readback(i-1); p1(i)+AllGather(i); tail(i-2) [y2 sum, step 6, out];
    mid(i-1) [steps 3-5 + y2 AllGather]. Every consumer is thus >= one
    full rep behind its producer, so no engine queue ever stalls on a
    collective or cross-engine dependency, and collective latency hides
    under the next rep's phase 1.
  All matmuls accumulate in fp32 PSUM; weights/activations bf16 or fp8.
"""

import numpy as np
import ml_dtypes

B, S, V, E = 32, 128, 8192, 256
NC = 8
BL = B // NC    # 4 local batches
ES = E // NC    # 32 feature shard (steps 3-5)
VS = V // NC    # 1024 vocab shard (step 6)
NCOL = BL * S   # 512 columns of local hT
NV = V // 128   # 64 v-chunks
GHT = 2 * 128 * NCOL          # bf16 elements of hT in gather payload
GLEN = GHT + NCOL             # + flattened s row

WEMB_SCALE = 64.0
XCH = 16        # v-chunks per x/w_emb DMA chunk (16 -> 1MB fp8 x chunks)
NVD = 32        # v-chunks computed as DoubleRow fp8e4 pairs (first half of V)

_CACHE: dict = {}


def _build_nc(reps: int = 1, stop_after: str = "all", skip_cc: bool = False,
              no_rb: bool = False, no_gin: bool = False):
    import concourse.bacc as bacc
    import concourse.mybir as mybir
    import concourse.tile as tile

    bf = mybir.dt.bfloat16
    f32 = mybir.dt.float32
    fp8 = mybir.dt.float8e3
    AF = mybir.ActivationFunctionType
    ALU = mybir.AluOpType

    nc = bacc.Bacc("TRN2", target_bir_lowering=False, debug=False, num_devices=NC)

    xt = nc.dram_tensor("xt", [128, NV * NCOL], fp8, kind="ExternalInput")
    wemb = nc.dram_tensor("wemb", [128, NV * E], fp8, kind="ExternalInput")
    bembe = nc.dram_tensor("bembe", [128, 2], f32, kind="ExternalInput")
    wr = nc.dram_tensor("wr", [128, ES * E], bf, kind="ExternalInput")
    bredr = nc.dram_tensor("bredr", [1, 16 * ES], bf, kind="ExternalInput")
    w2 = nc.dram_tensor("w2", [128, ES * E], bf, kind="ExternalInput")
    bred2 = nc.dram_tensor("bred2", [128, 2], f32, kind="ExternalInput")
    wo = nc.dram_tensor("wo", [128, 2 * VS], bf, kind="ExternalInput")
    boutr = nc.dram_tensor("boutr", [1, VS], bf, kind="ExternalInput")
    ones = nc.dram_tensor("ones", [128, 1], bf, kind="ExternalInput")
    onesr = nc.dram_tensor("onesr", [1, 128], bf, kind="ExternalInput")
    out_ext = nc.dram_tensor("out", [B, VS], f32, kind="ExternalOutput")

    # gather buffers; gout parity-double-buffered across pipelined reps
    gin = nc.dram_tensor("gin", [GLEN], bf)
    gout = [nc.dram_tensor(f"gout{p}", [NC, GLEN], bf, addr_space="Shared")
            for p in range(2)]
    s_off = GHT
    y2g_in = nc.dram_tensor("y2g_in", [B, E], f32)
    y2g_out = [nc.dram_tensor(f"y2g_out{p}", [NC * B, E], f32,
                              addr_space="Shared") for p in range(2)]

    groups = [list(range(NC))]
    STAGES = {"p1": 1, "gather": 2, "p3": 3, "p4": 3, "p5": 3, "ar": 3,
              "tail1": 4, "tail2": 4, "all": 4}
    stage = STAGES[stop_after]

    with tile.TileContext(nc) as tc:
        with (
            tc.tile_pool(name="persist", bufs=1) as pp,
            tc.tile_pool(name="wts", bufs=2) as wp,
            tc.tile_pool(name="wo3", bufs=3) as wp3,
            tc.tile_pool(name="xload", bufs=3) as xpool,
            tc.tile_pool(name="psum", bufs=1, space="PSUM") as psp,
        ):
            # ---------- persistent SBUF ----------
            hT_all = pp.tile([128, 2 * B * S], bf)       # [j128, (jc, b, s)]
            sT_all = pp.tile([128, B], bf)               # [k, (c, bl)]
            weff = pp.tile([128, 2 * ES * B], bf)        # [j128, (jc, e, b)]
            y1 = pp.tile([128, B * ES], bf)              # [k, (b, j)]
            hsb = pp.tile([128, 2 * NCOL], bf)           # local hT [j128, (jc, n)]
            s_bf = pp.tile([1, NCOL], bf)
            bembe_sb = pp.tile([128, 2], f32)
            bredr_sb = pp.tile([1, 16 * ES], bf)
            bred2_sb = pp.tile([128, 2], f32)
            ones_sb = pp.tile([128, 1], bf)
            onesr_sb = pp.tile([1, 128], bf)
            y2parts = pp.tile([B, NC * E], f32)          # [b, (c, e)]
            y2sum = pp.tile([B, E], f32)
            y2Tf = pp.tile([128, 2 * B], f32)            # [e128, (ec, b)]
            y2T = pp.tile([128, 2 * B], bf)
            boutr_sb = pp.tile([1, VS], bf)
            outsb = pp.tile([B, VS], f32)

            nc.sync.dma_start(bembe_sb[:, :], bembe[:, :])
            nc.sync.dma_start(bred2_sb[:, :], bred2[:, :])
            nc.sync.dma_start(ones_sb[:, :], ones[:, :])
            nc.sync.dma_start(onesr_sb[:, :], onesr[:, :])
            nc.sync.dma_start(bredr_sb[:, :], bredr[:, :])
            nc.sync.dma_start(boutr_sb[:, :], boutr[:, :])

            weff_v = weff.rearrange("p (jc e b) -> p jc e b", jc=2, e=ES)
            weff_p = weff.rearrange("p (jc e b) -> p e jc b", jc=2, e=ES)
            y1_v = y1.rearrange("p (b j) -> p b j", b=B)
            sT_v = sT_all.rearrange("p (c bl) -> p c bl", c=NC)

            def emit_p1(i):
                """phase 1 + gather issue for rep i (stage >= 1/2)."""
                wemb_res = wp.tile([128, NV * E], fp8, tag="wemb")
                wr_res = wp.tile([128, ES * E], bf, tag="wr")
                w2_res = wp.tile([128, ES * E], bf, tag="w2")
                wo_sb = wp3.tile([128, 2 * VS], bf, tag="wo")
                ph0 = psp.tile([128, NCOL], f32, tag="ph0", name="ph0")
                ph1 = psp.tile([128, NCOL], f32, tag="ph1", name="ph1")
                ps = psp.tile([1, NCOL], f32, tag="ps", name="ps")
                phs = [ph0, ph1]
                fp8e4 = mybir.dt.float8e4
                w4 = wemb_res.bitcast(fp8e4).rearrange(
                    "p (vp two e) -> p vp two e", two=2, e=E)
                for g in range(NV // XCH):
                    xt_t = xpool.tile([128, XCH * NCOL], fp8, tag="xt", name="xt_t")
                    nc.sync.dma_start(
                        xt_t[:, :], xt[:, g * XCH * NCOL:(g + 1) * XCH * NCOL]
                    )
                    nc.sync.dma_start(
                        wemb_res[:, g * XCH * E:(g + 1) * XCH * E],
                        wemb[:, g * XCH * E:(g + 1) * XCH * E],
                    )
                    if g < NVD // XCH:
                        # DoubleRow fp8e4: 256-deep contraction per matmul
                        x4 = xt_t.bitcast(fp8e4).rearrange(
                            "p (vp two n) -> p vp two n", two=2, n=NCOL)
                        for vpl in range(XCH // 2):
                            vp = g * (XCH // 2) + vpl
                            for ec in range(2):
                                nc.tensor.matmul(
                                    phs[ec][:, :],
                                    w4[:, vp, :, ec * 128:(ec + 1) * 128],
                                    x4[:, vpl, :, :],
                                    start=(vp == 0),
                                    stop=False,
                                    perf_mode=mybir.MatmulPerfMode.DoubleRow,
                                )
                    else:
                        for vci in range(XCH):
                            vc = g * XCH + vci
                            for ec in range(2):
                                nc.tensor.matmul(
                                    phs[ec][:, :],
                                    wemb_res[:, vc * E + ec * 128: vc * E + (ec + 1) * 128],
                                    xt_t[:, vci * NCOL:(vci + 1) * NCOL],
                                    start=False,
                                    stop=(vc == NV - 1),
                                )
                if stage >= 3:
                    nc.sync.dma_start(wr_res[:, 0:ES * E // 2], wr[:, 0:ES * E // 2])
                    nc.sync.dma_start(wr_res[:, ES * E // 2:], wr[:, ES * E // 2:])
                    nc.sync.dma_start(w2_res[:, 0:ES * E // 2], w2[:, 0:ES * E // 2])
                    nc.sync.dma_start(w2_res[:, ES * E // 2:], w2[:, ES * E // 2:])
                if stage >= 4:
                    nc.sync.dma_start(
                        wo_sb.rearrange("p (ec v) -> p ec v", ec=2),
                        wo.ap().rearrange("p (ec v) -> p ec v", ec=2),
                    )
                for ec in range(2):
                    nc.scalar.activation(
                        hsb[:, ec * NCOL:(ec + 1) * NCOL],
                        phs[ec][:, :],
                        AF.Relu,
                        bias=bembe_sb[:, ec:ec + 1],
                        scale=1.0 / WEMB_SCALE,
                    )
                for ec in range(2):
                    nc.tensor.matmul(
                        ps[:, :],
                        ones_sb[:, 0:1],
                        hsb[:, ec * NCOL:(ec + 1) * NCOL],
                        start=(ec == 0),
                        stop=(ec == 1),
                    )
                nc.vector.tensor_copy(s_bf[:, :], ps[:, :])
                if stage < 2 or no_gin:
                    return (wr_res, w2_res, wo_sb)
                # merged AllGather of (hT, s)
                nc.gpsimd.dma_start(
                    gin.ap()[0:GHT].rearrange("(jc p n) -> p jc n", jc=2, p=128),
                    hsb.rearrange("p (jc n) -> p jc n", jc=2),
                )
                nc.gpsimd.dma_start(
                    gin.ap()[s_off:s_off + NCOL].rearrange("(one n) -> one n",
                                                           one=1),
                    s_bf[:, :],
                )
                go = gout[i % 2]
                if skip_cc:
                    nc.gpsimd.dma_start(go.ap()[0], gin.ap()[:])
                else:
                    nc.gpsimd.collective_compute(
                        "AllGather", ALU.bypass, groups,
                        ins=[gin.ap().opt()], outs=[go.ap().opt()],
                    )
                return (wr_res, w2_res, wo_sb)

            def emit_rb(i):
                """readback of rep i's gather into SBUF (stage >= 2)."""
                if no_rb and i > 0:
                    return
                go = gout[i % 2]
                g_s = go.ap()[:, s_off:s_off + NCOL].rearrange(
                    "c (bl k) -> k c bl", bl=BL
                )
                for bl in range(BL):
                    nc.gpsimd.dma_start(sT_v[:, :, bl], g_s[:, :, bl])
                for jc in range(2):
                    nc.gpsimd.dma_start(
                        hT_all[:, jc * B * S:(jc + 1) * B * S].rearrange(
                            "p (c n) -> p c n", c=NC
                        ),
                        go.ap()[:, jc * 128 * NCOL:(jc + 1) * 128 * NCOL].rearrange(
                            "c (p n) -> p c n", p=128
                        ),
                    )

            def emit_mid(i, wts):
                """steps 3-5 + y2 AllGather for rep i (stage >= 3)."""
                wr_res, w2_res, _ = wts
                for g in range(ES // 8):
                    pw8 = psp.tile([128, 512], f32, tag="p32", bufs=2, name="pw8")
                    for e8 in range(8):
                        el = g * 8 + e8
                        for jc in range(2):
                            off = e8 * 64 + jc * 32
                            nc.tensor.matmul(
                                pw8[:, off:off + 32],
                                wr_res[:, el * E + jc * 128: el * E + (jc + 1) * 128],
                                sT_all[:, :],
                            )
                    nc.vector.tensor_copy(
                        weff_p[:, g * 8:(g + 1) * 8, :, :],
                        pw8.rearrange("p (e jc b) -> p e jc b", e=8, jc=2),
                    )
                for g in range(B // 16):
                    py16 = psp.tile([128, 512], f32, tag="p32", bufs=2, name="py16")
                    for bl in range(16):
                        b = g * 16 + bl
                        for jc in range(2):
                            nc.tensor.matmul(
                                py16[:, bl * ES:(bl + 1) * ES],
                                hT_all[:, jc * B * S + b * S: jc * B * S + (b + 1) * S],
                                weff_v[:, jc, :, b],
                                start=(jc == 0),
                                stop=False,
                            )
                        nc.tensor.matmul(
                            py16[:, bl * ES:(bl + 1) * ES],
                            onesr_sb[0:1, 0:128],
                            bredr_sb[0:1, bl * ES:(bl + 1) * ES],
                            start=False,
                            stop=True,
                        )
                    nc.scalar.activation(
                        y1[:, g * 512:(g + 1) * 512], py16[:, :], AF.Relu
                    )
                py2 = psp.tile([B, E], f32, tag="py2", name="py2")
                for jl in range(ES):
                    nc.tensor.matmul(
                        py2[:, :],
                        y1_v[:, :, jl],
                        w2_res[:, jl * E:(jl + 1) * E],
                        start=(jl == 0),
                        stop=(jl == ES - 1),
                    )
                nc.vector.tensor_copy(y2sum[:, 0:E], py2[:, :])
                nc.gpsimd.dma_start(y2g_in[:, :], y2sum[:, 0:E])
                yo = y2g_out[i % 2]
                if skip_cc:
                    for c in range(NC):
                        nc.gpsimd.dma_start(
                            yo.ap()[c * B:(c + 1) * B, :], y2g_in[:, :]
                        )
                else:
                    nc.gpsimd.collective_compute(
                        "AllGather", ALU.bypass, groups,
                        ins=[y2g_in.ap().opt()], outs=[yo.ap().opt()],
                    )

            def emit_tail(i, wts):
                """y2 reduce + step 6 + output for rep i (stage >= 4)."""
                _, _, wo_sb = wts
                yo = y2g_out[i % 2]
                nc.gpsimd.dma_start(
                    y2parts.rearrange("b (c e) -> b c e", c=NC),
                    yo.ap().rearrange("(c b) e -> b c e", c=NC),
                )
                nc.vector.tensor_tensor(
                    y2parts[:, 0:4 * E], y2parts[:, 0:4 * E],
                    y2parts[:, 4 * E:8 * E], ALU.add,
                )
                nc.vector.tensor_tensor(
                    y2parts[:, 0:2 * E], y2parts[:, 0:2 * E],
                    y2parts[:, 2 * E:4 * E], ALU.add,
                )
                nc.vector.tensor_tensor(
                    y2sum[:, 0:E], y2parts[:, 0:E],
                    y2parts[:, E:2 * E], ALU.add,
                )
                for ec in range(2):
                    for j4 in range(4):
                        nc.vector.transpose(
                            y2Tf[j4 * 32:(j4 + 1) * 32, ec * B:(ec + 1) * B],
                            y2sum[:, ec * 128 + j4 * 32: ec * 128 + (j4 + 1) * 32],
                        )
                    nc.scalar.activation(
                        y2T[:, ec * B:(ec + 1) * B],
                        y2Tf[:, ec * B:(ec + 1) * B],
                        AF.Relu,
                        bias=bred2_sb[:, ec:ec + 1],
                    )
                for nv in range(2):
                    pso = psp.tile([B, 512], f32, tag="po", bufs=2, name="pso")
                    for ec in range(2):
                        nc.tensor.matmul(
                            pso[:, :],
                            y2T[:, ec * B:(ec + 1) * B],
                            wo_sb[:, ec * VS + nv * 512: ec * VS + (nv + 1) * 512],
                            start=(ec == 0),
                            stop=False,
                        )
                    nc.tensor.matmul(
                        pso[:, :],
                        onesr_sb[0:1, 0:B],
                        boutr_sb[0:1, nv * 512:(nv + 1) * 512],
                        start=False,
                        stop=True,
                    )
                    nc.vector.tensor_copy(outsb[:, nv * 512:(nv + 1) * 512], pso[:, :])
                nc.gpsimd.dma_start(out_ext[:, :], outsb[:, :])

            # ---- 2-deep software pipeline over reps ----
            wts_hist: dict = {}
            for i in range(reps):
                if stage >= 2 and i >= 1:
                    emit_rb(i - 1)
                wts_hist[i] = emit_p1(i)
                if stage >= 4 and i >= 2:
                    emit_tail(i - 2, wts_hist.pop(i - 2))
                if stage >= 3 and i >= 1:
                    emit_mid(i - 1, wts_hist[i - 1])
            # drain
            if stage >= 2 and reps >= 1:
                emit_rb(reps - 1)
            if stage >= 3 and reps >= 1:
                emit_mid(reps - 1, wts_hist[reps - 1])
            if stage >= 4:
                if reps >= 2:
                    emit_tail(reps - 2, wts_hist.pop(reps - 2))
                emit_tail(reps - 1, wts_hist.pop(reps - 1))

    nc.compile()
    return nc


def _get_nc():
    if "nc" not in _CACHE:
        _CACHE["nc"] = _build_nc()
    return _CACHE["nc"]


def _pm(a):
    """[V-like rows, cols] -> partition-major [128, (chunks, cols)]."""
    v, c = a.shape
    return np.ascontiguousarray(
        a.reshape(v // 128, 128, c).transpose(1, 0, 2).reshape(128, -1)
    )


def _pack_inputs(x, w_emb, b_emb, w_red, b_red, w_red2, b_red2, w_out, b_out):
    bf = ml_dtypes.bfloat16
    fp8 = ml_dtypes.float8_e3m4
    f32 = np.float32

    fp8e4 = ml_dtypes.float8_e4m3

    def _mixq(a_f32, ncols_dr):
        dr = np.ascontiguousarray(a_f32[:, :ncols_dr]).astype(fp8e4).view(fp8)
        e3 = np.ascontiguousarray(a_f32[:, ncols_dr:]).astype(fp8)
        return np.ascontiguousarray(np.concatenate([dr, e3], axis=1))

    x = np.asarray(x, f32)
    w_emb = np.asarray(w_emb, f32)
    wembT = _mixq(_pm(np.ascontiguousarray(w_emb.T) * WEMB_SCALE), NVD * E)
    bemb_eff = (np.asarray(b_emb, np.float64)
                + 0.5 * np.asarray(w_emb, np.float64).sum(axis=1)).astype(f32)
    bembe = np.ascontiguousarray(bemb_eff.reshape(2, 128).T)         # [128, 2]
    Wr = np.asarray(w_red, f32).reshape(E, S, E)                     # [e, k, j]
    W2 = np.asarray(w_red2, f32).reshape(E, S, E)                    # [eo, k, j]
    woT = np.ascontiguousarray(np.asarray(w_out, f32).T)             # [E, V]
    bred2c = np.ascontiguousarray(
        np.asarray(b_red2, f32).reshape(2, 128).T)                   # [128, 2]
    ones = np.ones((128, 1), dtype=bf)
    onesr = np.ones((1, 128), dtype=bf)

    in_maps = []
    for c in range(NC):
        xs = np.asarray(x[c * BL:(c + 1) * BL])                      # [4, S, V]
        xc = xs.transpose(2, 0, 1).reshape(V, NCOL) - 0.5            # [V, 512]
        xt = _mixq(_pm(xc), NVD * NCOL)                              # [128,(vc,n)]
        wr_c = np.ascontiguousarray(
            Wr[c * ES:(c + 1) * ES].transpose(1, 0, 2).reshape(S, ES * E)
        ).astype(bf)
        w2_c = np.ascontiguousarray(
            W2[:, :, c * ES:(c + 1) * ES].transpose(1, 2, 0).reshape(S, ES * E)
        ).astype(bf)
        wo_c = _pm(woT[:, c * VS:(c + 1) * VS]).astype(bf)           # [128,(ec,v)]
        bredr = np.tile(b_red[c * ES:(c + 1) * ES], 16).reshape(1, 16 * ES).astype(bf)
        boutr = np.asarray(b_out[c * VS:(c + 1) * VS]).reshape(1, VS).astype(bf)
        in_maps.append({
            "xt": xt, "wemb": wembT, "bembe": bembe,
            "wr": wr_c, "bredr": bredr,
            "w2": w2_c, "bred2": bred2c,
            "wo": wo_c, "boutr": boutr,
            "ones": ones, "onesr": onesr,
        })
    return in_maps


def kernel(x, w_emb, b_emb, w_red, b_red, w_red2, b_red2, w_out, b_out):
    from concourse.bass_utils import run_bass_kernel_spmd

    nc = _get_nc()
    x, w_emb, b_emb, w_red, b_red, w_red2, b_red2, w_out, b_out = (
        np.asarray(a, dtype=np.float32)
        for a in (x, w_emb, b_emb, w_red, b_red, w_red2, b_red2, w_out, b_out)
    )
    in_maps = _pack_inputs(x, w_emb, b_emb, w_red, b_red, w_red2, b_red2, w_out, b_out)
    res = run_bass_kernel_spmd(nc, in_maps, core_ids=list(range(NC)))
    out = np.concatenate([res.results[c]["out"] for c in range(NC)], axis=1)
    return np.ascontiguousarray(out, dtype=np.float32)


# revision 20
# speedup vs baseline: 1.0022x; 1.0022x over previous
"""Trainium2 Bass kernel for nn_CrossBaby_1 (B=32, S=128, V=8192, E=256).

Strategy (8 NeuronCores, single NEFF, collectives, 2-deep software pipeline):
  - Step 1 (x @ w_emb.T, 17 GFLOP): data-parallel over batch. Each core
    computes hT for its 4 batches from a host-pretransposed, centered
    (x-0.5) shard in fp8-e3m4. w_emb is also fp8-e3m4, host-scaled by 64
    (undone by the activation's scale). x + w_emb stream in lockstep on
    the sync HWDGE ring; wr/w2/wo stream right behind them, all during
    phase 1's compute. Dependent DMAs (gather bounce, readbacks, output)
    live on the gpsimd (SWDGE) queue so the input ring never stalls.
  - ONE merged AllGather of (hT bf16 + row-sums s) across the 8 cores.
  - Steps 3-5 (w_red / w_red2): tensor-parallel over the e/j feature dim.
  - y2 partial reduction: AllGather + local DVE tree-sum.
  - Step 6 (w_out): tensor-parallel over vocab; host concatenates.
  - Bias adds for steps 4/6 are folded into the PSUM accumulation as
    rank-1 matmuls (ones ⊗ bias_row).
  - The rep loop is software-pipelined two deep: body(i) emits
    readback(i-1); p1(i)+AllGather(i); tail(i-2) [y2 sum, step 6, out];
    mid(i-1) [steps 3-5 + y2 AllGather]. Every consumer is thus >= one
    full rep behind its producer, so no engine queue ever stalls on a
    collective or cross-engine dependency, and collective latency hides
    under the next rep's phase 1.
  All matmuls accumulate in fp32 PSUM; weights/activations bf16 or fp8.
"""

import numpy as np
import ml_dtypes

B, S, V, E = 32, 128, 8192, 256
NC = 8
BL = B // NC    # 4 local batches
ES = E // NC    # 32 feature shard (steps 3-5)
VS = V // NC    # 1024 vocab shard (step 6)
NCOL = BL * S   # 512 columns of local hT
NV = V // 128   # 64 v-chunks
GHT = 2 * 128 * NCOL          # bf16 elements of hT in gather payload
GLEN = GHT + NCOL             # + flattened s row

WEMB_SCALE = 64.0
XCH = 16        # v-chunks per x/w_emb DMA chunk (16 -> 1MB fp8 x chunks)
NVD = 32        # v-chunks computed as DoubleRow fp8e4 pairs (first half of V)

_CACHE: dict = {}


def _build_nc(reps: int = 1, stop_after: str = "all", skip_cc: bool = False,
              no_rb: bool = False, no_gin: bool = False):
    import concourse.bacc as bacc
    import concourse.mybir as mybir
    import concourse.tile as tile

    bf = mybir.dt.bfloat16
    f32 = mybir.dt.float32
    fp8 = mybir.dt.float8e3
    AF = mybir.ActivationFunctionType
    ALU = mybir.AluOpType

    nc = bacc.Bacc("TRN2", target_bir_lowering=False, debug=False, num_devices=NC)

    xt = nc.dram_tensor("xt", [128, NV * NCOL], fp8, kind="ExternalInput")
    wemb = nc.dram_tensor("wemb", [128, NV * E], fp8, kind="ExternalInput")
    bembe = nc.dram_tensor("bembe", [128, 2], f32, kind="ExternalInput")
    wr = nc.dram_tensor("wr", [128, ES * E], bf, kind="ExternalInput")
    bredr = nc.dram_tensor("bredr", [1, 16 * ES], bf, kind="ExternalInput")
    w2 = nc.dram_tensor("w2", [128, ES * E], bf, kind="ExternalInput")
    bred2 = nc.dram_tensor("bred2", [128, 2], f32, kind="ExternalInput")
    wo = nc.dram_tensor("wo", [128, 2 * VS], bf, kind="ExternalInput")
    boutr = nc.dram_tensor("boutr", [1, VS], bf, kind="ExternalInput")
    ones = nc.dram_tensor("ones", [128, 1], bf, kind="ExternalInput")
    onesr = nc.dram_tensor("onesr", [1, 128], bf, kind="ExternalInput")
    out_ext = nc.dram_tensor("out", [B, VS], f32, kind="ExternalOutput")

    # gather buffers; gout parity-double-buffered across pipelined reps
    gin = nc.dram_tensor("gin", [GLEN], bf)
    gout = [nc.dram_tensor(f"gout{p}", [NC, GLEN], bf, addr_space="Shared")
            for p in range(2)]
    s_off = GHT
    y2g_in = nc.dram_tensor("y2g_in", [B, E], f32)
    y2g_out = [nc.dram_tensor(f"y2g_out{p}", [NC * B, E], f32,
                              addr_space="Shared") for p in range(2)]

    groups = [list(range(NC))]
    STAGES = {"p1": 1, "gather": 2, "p3": 3, "p4": 3, "p5": 3, "ar": 3,
              "tail1": 4, "tail2": 4, "all": 4}
    stage = STAGES[stop_after]

    with tile.TileContext(nc) as tc:
        with (
            tc.tile_pool(name="persist", bufs=1) as pp,
            tc.tile_pool(name="wts", bufs=2) as wp,
            tc.tile_pool(name="wo3", bufs=3) as wp3,
            tc.tile_pool(name="xload", bufs=3) as xpool,
            tc.tile_pool(name="psum", bufs=1, space="PSUM") as psp,
        ):
            # ---------- persistent SBUF ----------
            hT_all = pp.tile([128, 2 * B * S], bf)       # [j128, (jc, b, s)]
            sT_all = pp.tile([128, B], bf)               # [k, (c, bl)]
            weff = pp.tile([128, 2 * ES * B], bf)        # [j128, (jc, e, b)]
            y1 = pp.tile([128, B * ES], bf)              # [k, (b, j)]
            hsb = pp.tile([128, 2 * NCOL], bf)           # local hT [j128, (jc, n)]
            s_bf = pp.tile([1, NCOL], bf)
            bembe_sb = pp.tile([128, 2], f32)
            bredr_sb = pp.tile([1, 16 * ES], bf)
            bred2_sb = pp.tile([128, 2], f32)
            ones_sb = pp.tile([128, 1], bf)
            onesr_sb = pp.tile([1, 128], bf)
            y2parts = pp.tile([B, NC * E], f32)          # [b, (c, e)]
            y2sum = pp.tile([B, E], f32)
            y2Tf = pp.tile([128, 2 * B], f32)            # [e128, (ec, b)]
            y2T = pp.tile([128, 2 * B], bf)
            boutr_sb = pp.tile([1, VS], bf)
            outsb = pp.tile([B, VS], f32)

            nc.sync.dma_start(bembe_sb[:, :], bembe[:, :])
            nc.sync.dma_start(bred2_sb[:, :], bred2[:, :])
            nc.sync.dma_start(ones_sb[:, :], ones[:, :])
            nc.sync.dma_start(onesr_sb[:, :], onesr[:, :])
            nc.sync.dma_start(bredr_sb[:, :], bredr[:, :])
            nc.sync.dma_start(boutr_sb[:, :], boutr[:, :])

            weff_v = weff.rearrange("p (jc e b) -> p jc e b", jc=2, e=ES)
            weff_p = weff.rearrange("p (jc e b) -> p e jc b", jc=2, e=ES)
            y1_v = y1.rearrange("p (b j) -> p b j", b=B)
            sT_v = sT_all.rearrange("p (c bl) -> p c bl", c=NC)

            def emit_p1(i):
                """phase 1 + gather issue for rep i (stage >= 1/2)."""
                wemb_res = wp.tile([128, NV * E], fp8, tag="wemb")
                wr_res = wp.tile([128, ES * E], bf, tag="wr")
                w2_res = wp.tile([128, ES * E], bf, tag="w2")
                wo_sb = wp3.tile([128, 2 * VS], bf, tag="wo")
                ph0 = psp.tile([128, NCOL], f32, tag="ph0", name="ph0")
                ph1 = psp.tile([128, NCOL], f32, tag="ph1", name="ph1")
                ps = psp.tile([1, NCOL], f32, tag="ps", name="ps")
                phs = [ph0, ph1]
                fp8e4 = mybir.dt.float8e4
                w4 = wemb_res.bitcast(fp8e4).rearrange(
                    "p (vp two e) -> p vp two e", two=2, e=E)
                for g in range(NV // XCH):
                    xt_t = xpool.tile([128, XCH * NCOL], fp8, tag="xt", name="xt_t")
                    nc.sync.dma_start(
                        xt_t[:, :], xt[:, g * XCH * NCOL:(g + 1) * XCH * NCOL]
                    )
                    nc.sync.dma_start(
                        wemb_res[:, g * XCH * E:(g + 1) * XCH * E],
                        wemb[:, g * XCH * E:(g + 1) * XCH * E],
                    )
                    if g < NVD // XCH:
                        # DoubleRow fp8e4: 256-deep contraction per matmul
                        x4 = xt_t.bitcast(fp8e4).rearrange(
                            "p (vp two n) -> p vp two n", two=2, n=NCOL)
                        for vpl in range(XCH // 2):
                            vp = g * (XCH // 2) + vpl
                            for ec in range(2):
                                nc.tensor.matmul(
                                    phs[ec][:, :],
                                    w4[:, vp, :, ec * 128:(ec + 1) * 128],
                                    x4[:, vpl, :, :],
                                    start=(vp == 0),
                                    stop=False,
                                    perf_mode=mybir.MatmulPerfMode.DoubleRow,
                                )
                    else:
                        for vci in range(XCH):
                            vc = g * XCH + vci
                            for ec in range(2):
                                nc.tensor.matmul(
                                    phs[ec][:, :],
                                    wemb_res[:, vc * E + ec * 128: vc * E + (ec + 1) * 128],
                                    xt_t[:, vci * NCOL:(vci + 1) * NCOL],
                                    start=False,
                                    stop=(vc == NV - 1),
                                )
                if stage >= 3:
                    nc.sync.dma_start(wr_res[:, 0:ES * E // 2], wr[:, 0:ES * E // 2])
                    nc.sync.dma_start(wr_res[:, ES * E // 2:], wr[:, ES * E // 2:])
                    nc.sync.dma_start(w2_res[:, 0:ES * E // 2], w2[:, 0:ES * E // 2])
                    nc.sync.dma_start(w2_res[:, ES * E // 2:], w2[:, ES * E // 2:])
                if stage >= 4:
                    nc.sync.dma_start(
                        wo_sb.rearrange("p (ec v) -> p ec v", ec=2),
                        wo.ap().rearrange("p (ec v) -> p ec v", ec=2),
                    )
                for ec in range(2):
                    nc.scalar.activation(
                        hsb[:, ec * NCOL:(ec + 1) * NCOL],
                        phs[ec][:, :],
                        AF.Relu,
                        bias=bembe_sb[:, ec:ec + 1],
                        scale=1.0 / WEMB_SCALE,
                    )
                for ec in range(2):
                    nc.tensor.matmul(
                        ps[:, :],
                        ones_sb[:, 0:1],
                        hsb[:, ec * NCOL:(ec + 1) * NCOL],
                        start=(ec == 0),
                        stop=(ec == 1),
                    )
                nc.vector.tensor_copy(s_bf[:, :], ps[:, :])
                if stage < 2 or no_gin:
                    return (wr_res, w2_res, wo_sb)
                # merged AllGather of (hT, s)
                nc.gpsimd.dma_start(
                    gin.ap()[0:GHT].rearrange("(jc p n) -> p jc n", jc=2, p=128),
                    hsb.rearrange("p (jc n) -> p jc n", jc=2),
                )
                nc.gpsimd.dma_start(
                    gin.ap()[s_off:s_off + NCOL].rearrange("(one n) -> one n",
                                                           one=1),
                    s_bf[:, :],
                )
                go = gout[i % 2]
                if skip_cc:
                    nc.gpsimd.dma_start(go.ap()[0], gin.ap()[:])
                else:
                    nc.gpsimd.collective_compute(
                        "AllGather", ALU.bypass, groups,
                        ins=[gin.ap().opt()], outs=[go.ap().opt()],
                    )
                return (wr_res, w2_res, wo_sb)

            def emit_rb(i):
                """readback of rep i's gather into SBUF (stage >= 2)."""
                if no_rb and i > 0:
                    return
                go = gout[i % 2]
                g_s = go.ap()[:, s_off:s_off + NCOL].rearrange(
                    "c (bl k) -> k c bl", bl=BL
                )
                for bl in range(BL):
                    nc.gpsimd.dma_start(sT_v[:, :, bl], g_s[:, :, bl])
                for jc in range(2):
                    nc.gpsimd.dma_start(
                        hT_all[:, jc * B * S:(jc + 1) * B * S].rearrange(
                            "p (c n) -> p c n", c=NC
                        ),
                        go.ap()[:, jc * 128 * NCOL:(jc + 1) * 128 * NCOL].rearrange(
                            "c (p n) -> p c n", p=128
                        ),
                    )

            def emit_mid(i, wts):
                """steps 3-5 + y2 AllGather for rep i (stage >= 3)."""
                wr_res, w2_res, _ = wts
                for g in range(ES // 8):
                    pw8 = psp.tile([128, 512], f32, tag="p32", bufs=2, name="pw8")
                    for e8 in range(8):
                        el = g * 8 + e8
                        for jc in range(2):
                            off = e8 * 64 + jc * 32
                            nc.tensor.matmul(
                                pw8[:, off:off + 32],
                                wr_res[:, el * E + jc * 128: el * E + (jc + 1) * 128],
                                sT_all[:, :],
                            )
                    nc.vector.tensor_copy(
                        weff_p[:, g * 8:(g + 1) * 8, :, :],
                        pw8.rearrange("p (e jc b) -> p e jc b", e=8, jc=2),
                    )
                for g in range(B // 16):
                    py16 = psp.tile([128, 512], f32, tag="p32", bufs=2, name="py16")
                    for bl in range(16):
                        b = g * 16 + bl
                        for jc in range(2):
                            nc.tensor.matmul(
                                py16[:, bl * ES:(bl + 1) * ES],
                                hT_all[:, jc * B * S + b * S: jc * B * S + (b + 1) * S],
                                weff_v[:, jc, :, b],
                                start=(jc == 0),
                                stop=False,
                            )
                        nc.tensor.matmul(
                            py16[:, bl * ES:(bl + 1) * ES],
                            onesr_sb[0:1, 0:128],
                            bredr_sb[0:1, bl * ES:(bl + 1) * ES],
                            start=False,
                            stop=True,
                        )
                    nc.scalar.activation(
                        y1[:, g * 512:(g + 1) * 512], py16[:, :], AF.Relu
                    )
                py2 = psp.tile([B, E], f32, tag="py2", name="py2")
                for jl in range(ES):
                    nc.tensor.matmul(
                        py2[:, :],
                        y1_v[:, :, jl],
                        w2_res[:, jl * E:(jl + 1) * E],
                        start=(jl == 0),
                        stop=(jl == ES - 1),
                    )
                nc.vector.tensor_copy(y2sum[:, 0:E], py2[:, :])
                nc.gpsimd.dma_start(y2g_in[:, :], y2sum[:, 0:E])
                yo = y2g_out[i % 2]
                if skip_cc:
                    for c in range(NC):
                        nc.gpsimd.dma_start(
                            yo.ap()[c * B:(c + 1) * B, :], y2g_in[:, :]
                        )
                else:
                    nc.gpsimd.collective_compute(
                        "AllGather", ALU.bypass, groups,
                        ins=[y2g_in.ap().opt()], outs=[yo.ap().opt()],
                    )

            def emit_tail(i, wts):
                """y2 reduce + step 6 + output for rep i (stage >= 4)."""
                _, _, wo_sb = wts
                yo = y2g_out[i % 2]
                nc.gpsimd.dma_start(
                    y2parts.rearrange("b (c e) -> b c e", c=NC),
                    yo.ap().rearrange("(c b) e -> b c e", c=NC),
                )
                nc.vector.tensor_tensor(
                    y2parts[:, 0:4 * E], y2parts[:, 0:4 * E],
                    y2parts[:, 4 * E:8 * E], ALU.add,
                )
                nc.vector.tensor_tensor(
                    y2parts[:, 0:2 * E], y2parts[:, 0:2 * E],
                    y2parts[:, 2 * E:4 * E], ALU.add,
                )
                nc.vector.tensor_tensor(
                    y2sum[:, 0:E], y2parts[:, 0:E],
                    y2parts[:, E:2 * E], ALU.add,
                )
                for ec in range(2):
                    for j4 in range(4):
                        nc.vector.transpose(
                            y2Tf[j4 * 32:(j4 + 1) * 32, ec * B:(ec + 1) * B],
                            y2sum[:, ec * 128 + j4 * 32: ec * 128 + (j4 + 1) * 32],
                        )
                    nc.scalar.activation(
                        y2T[:, ec * B:(ec + 1) * B],
                        y2Tf[:, ec * B:(ec + 1) * B],
                        AF.Relu,
                        bias=bred2_sb[:, ec:ec + 1],
                    )
                for nv in range(2):
                    pso = psp.tile([B, 512], f32, tag="po", bufs=2, name="pso")
                    for ec in range(2):
                        nc.tensor.matmul(
                            pso[:, :],
                            y2T[:, ec * B:(ec + 1) * B],
                            wo_sb[:, ec * VS + nv * 512: ec * VS + (nv + 1) * 512],
                            start=(ec == 0),
                            stop=False,
                        )
                    nc.tensor.matmul(
                        pso[:, :],
                        onesr_sb[0:1, 0:B],
                        boutr_sb[0:1, nv * 512:(nv + 1) * 512],
                        start=False,
                        stop=True,
                    )
                    nc.vector.tensor_copy(outsb[:, nv * 512:(nv + 1) * 512], pso[:, :])
                nc.gpsimd.dma_start(out_ext[:, :], outsb[:, :])

            # ---- 2-deep software pipeline over reps ----
            wts_hist: dict = {}
            for i in range(reps):
                if stage >= 2 and i >= 1:
                    emit_rb(i - 1)
                wts_hist[i] = emit_p1(i)
                if stage >= 4 and i >= 2:
                    emit_tail(i - 2, wts_hist.pop(i - 2))
                if stage >= 3 and i >= 1:
                    emit_mid(i - 1, wts_hist[i - 1])
            # drain
            if stage >= 2 and reps >= 1:
                emit_rb(reps - 1)
            if stage >= 3 and reps >= 1:
                emit_mid(reps - 1, wts_hist[reps - 1])
            if stage >= 4:
                if reps >= 2:
                    emit_tail(reps - 2, wts_hist.pop(reps - 2))
                emit_tail(reps - 1, wts_hist.pop(reps - 1))

    nc.compile()
    return nc


def _get_nc():
    if "nc" not in _CACHE:
        _CACHE["nc"] = _build_nc()
    return _CACHE["nc"]


def _pm(a):
    """[V-like rows, cols] -> partition-major [128, (chunks, cols)]."""
    v, c = a.shape
    return np.ascontiguousarray(
        a.reshape(v // 128, 128, c).transpose(1, 0, 2).reshape(128, -1)
    )


def _pack_inputs(x, w_emb, b_emb, w_red, b_red, w_red2, b_red2, w_out, b_out):
    bf = ml_dtypes.bfloat16
    fp8 = ml_dtypes.float8_e3m4
    f32 = np.float32

    fp8e4 = ml_dtypes.float8_e4m3

    def _mixq(a_f32, ncols_dr):
        dr = np.ascontiguousarray(a_f32[:, :ncols_dr]).astype(fp8e4).view(fp8)
        e3 = np.ascontiguousarray(a_f32[:, ncols_dr:]).astype(fp8)
        return np.ascontiguousarray(np.concatenate([dr, e3], axis=1))

    x = np.asarray(x, f32)
    w_emb = np.asarray(w_emb, f32)
    wembT = _mixq(_pm(np.ascontiguousarray(w_emb.T) * WEMB_SCALE), NVD * E)
    bemb_eff = (np.asarray(b_emb, np.float64)
                + 0.5 * np.asarray(w_emb, np.float64).sum(axis=1)).astype(f32)
    bembe = np.ascontiguousarray(bemb_eff.reshape(2, 128).T)         # [128, 2]
    Wr = np.asarray(w_red, f32).reshape(E, S, E)                     # [e, k, j]
    W2 = np.asarray(w_red2, f32).reshape(E, S, E)                    # [eo, k, j]
    woT = np.ascontiguousarray(np.asarray(w_out, f32).T)             # [E, V]
    bred2c = np.ascontiguousarray(
        np.asarray(b_red2, f32).reshape(2, 128).T)                   # [128, 2]
    ones = np.ones((128, 1), dtype=bf)
    onesr = np.ones((1, 128), dtype=bf)

    in_maps = []
    for c in range(NC):
        xs = np.asarray(x[c * BL:(c + 1) * BL])                      # [4, S, V]
        xc = xs.transpose(2, 0, 1).reshape(V, NCOL) - 0.5            # [V, 512]
        xt = _mixq(_pm(xc), NVD * NCOL)                              # [128,(vc,n)]
        wr_c = np.ascontiguousarray(
            Wr[c * ES:(c + 1) * ES].transpose(1, 0, 2).reshape(S, ES * E)
        ).astype(bf)
        w2_c = np.ascontiguousarray(
            W2[:, :, c * ES:(c + 1) * ES].transpose(1, 2, 0).reshape(S, ES * E)
        ).astype(bf)
        wo_c = _pm(woT[:, c * VS:(c + 1) * VS]).astype(bf)           # [128,(ec,v)]
        bredr = np.tile(b_red[c * ES:(c + 1) * ES], 16).reshape(1, 16 * ES).astype(bf)
        boutr = np.asarray(b_out[c * VS:(c + 1) * VS]).reshape(1, VS).astype(bf)
        in_maps.append({
            "xt": xt, "wemb": wembT, "bembe": bembe,
            "wr": wr_c, "bredr": bredr,
            "w2": w2_c, "bred2": bred2c,
            "wo": wo_c, "boutr": boutr,
            "ones": ones, "onesr": onesr,
        })
    return in_maps


def kernel(x, w_emb, b_emb, w_red, b_red, w_red2, b_red2, w_out, b_out):
    from concourse.bass_utils import run_bass_kernel_spmd

    nc = _get_nc()
    x, w_emb, b_emb, w_red, b_red, w_red2, b_red2, w_out, b_out = (
        np.asarray(a, dtype=np.float32)
        for a in (x, w_emb, b_emb, w_red, b_red, w_red2, b_red2, w_out, b_out)
    )
    in_maps = _pack_inputs(x, w_emb, b_emb, w_red, b_red, w_red2, b_red2, w_out, b_out)
    res = run_bass_kernel_spmd(nc, in_maps, core_ids=list(range(NC)))
    out = np.concatenate([res.results[c]["out"] for c in range(NC)], axis=1)
    return np.ascontiguousarray(out, dtype=np.float32)
